# revision 9
# baseline (speedup 1.0000x reference)
"""CTRNN cell (6 Euler unfolds) on 8 Trainium2 NeuronCores.

Math (per unfold, 6x):
    f     = tanh([x, s] @ W + b)
    s_new = s + 0.1 * (-s + f)  = 0.9*s + 0.1*f

Strategy (v4):
  - Data-parallel over batch: B=8192 -> 1024 rows/core, no cross-core
    communication. Host does the cheap numpy transposes/packing/casts.
  - All tensors kept TRANSPOSED on-chip (feature dim on SBUF partitions,
    batch on the free dim): state/x feed the tensor engine as the moving
    operand and W k-tile slices are directly the stationary lhsT.
  - Everything fp16 (except PSUM + final output f32). fp16 runs the PE
    at bf16 rate, gives 16-bit DVE ops the 2x packed perf mode, and its
    10-bit mantissa keeps the 5 accumulating state updates accurate:
    simulated end-to-end rel err 1.2e-3 (gate is 2e-2).
  - Delta-form matmuls: one persistent PSUM bank per (m-tile, chunk)
    holds z = x@Wt + s_k@Wb across all unfolds, updated with
    psum += (f_k - s_k) @ (0.1*Wb). 7-logical-matmul FLOP floor; PSUM
    never restarts. 0.1*Wb is host-prefolded. The init matmul feeds
    v0 = 10*s0 through 0.1*Wb (== s0@Wb), then the state tiles are
    rescaled in place to s (one free 2x DVE pass under the init
    matmuls) so the hot loop's critical op is the plain fp16 subtract
    tmp = f - s (2x mode, 408ns). The state update s += 0.1*tmp is a
    1x scalar_tensor_tensor but runs off the critical path, split
    2x DVE / 2x GpSimd per round.
  - Batch is split into 2 chunks of 512 that alternate on the PE: while
    chunk A runs its 16 delta matmuls, chunk B does tanh (ACT) + the
    tmp subtract (DVE) for the same unfold, and vice versa. The PE
    never waits out the ACT->DVE chain, so it stays busy (and the HAM
    clock gate stays at 2.4 GHz) through the steady state. All
    PSUM/state tiles are split per (m-tile, chunk) so Tile's
    tile-granular hazard tracking cannot serialize the two chunks.
  - Tail: u = 0.9*s_5 is precomputed under the last matmul round, so
    the final unfold is one fused op out = 0.1*f + u (split DVE/GpSimd)
    feeding 8 output DMAs balanced over the sync/scalar/gpsimd queues.
  - Inputs are host-packed (128, k*4KB) chunk-major so every DMA has
    >=2KB-contiguous per-partition runs, spread over the sync/scalar
    HWDGE rings and the gpsimd SWDGE path; the first x/W pieces are
    split so the init matmuls can start as early as possible.
  - A short junk-matmul warm-up bridges the PE from the NEFF preamble
    to the first data landing without delaying the real matmuls.
"""

import numpy as np

UNFOLDS = 6
DT = 0.1
B, D, N = 8192, 512, 512
NCORES = 8
BC = B // NCORES          # batch rows per core (1024)
CH = 512                  # chunk free-dim (one PSUM bank of f32)
NCH = BC // CH            # 2 chunks
P = 128
KT = D // P               # 4 k-tiles per operand half
MT = N // P               # 4 m-tiles of the output dim

_compiled_nc = None


def _build_nc():
    import concourse.bass as bass  # noqa: F401
    import concourse.bacc as bacc
    import concourse.tile as tile
    from concourse import mybir

    f32 = mybir.dt.float32
    f16 = mybir.dt.float16
    MULT = mybir.AluOpType.mult
    ADD = mybir.AluOpType.add
    SUB = mybir.AluOpType.subtract
    TANH = mybir.ActivationFunctionType.Tanh

    nc = bacc.Bacc("TRN2", target_bir_lowering=False, debug=False)

    xP = nc.dram_tensor("xP", [P, NCH * KT * CH], f16, kind="ExternalInput").ap()
    vP = nc.dram_tensor("vP", [P, NCH * KT * CH], f16, kind="ExternalInput").ap()
    WP = nc.dram_tensor("WP", [P, 2 * KT * N], f16, kind="ExternalInput").ap()
    bias = nc.dram_tensor("bias", [N], f32, kind="ExternalInput").ap()
    outT = nc.dram_tensor("outT", [N, BC], f32, kind="ExternalOutput").ap()

    with tile.TileContext(nc) as tc:
        with (
            tc.tile_pool(name="wpool", bufs=1) as wpool,
            tc.tile_pool(name="data", bufs=1) as data,
            tc.tile_pool(name="fpool", bufs=2) as fpool,
            tc.tile_pool(name="tmpp", bufs=2) as tmpp,
            tc.tile_pool(name="outp", bufs=1) as outp,
            tc.tile_pool(name="psum", bufs=1, space="PSUM") as psump,
        ):
            # warm-up junk tile first thing on the gpsimd queue so the
            # warm-up matmuls can start before any data lands
            junk = wpool.tile([P, N], f16, tag="junk", name="junk")
            nc.gpsimd.memset(junk[:], 0)

            # ---- input DMAs --------------------------------------------
            # first-needed first, split fine so the init matmuls start
            # as early as possible: Wt + x_A halves feed init chunk A.
            w_sb = wpool.tile([P, 2 * KT * N], f16, tag="w", name="w_sb")
            x_sb = data.tile([P, NCH * KT * CH], f16, tag="x", name="x_sb")
            v_t = [data.tile([P, KT * CH], f16, tag=f"v{c}", name=f"v{c}")
                   for c in range(NCH)]
            H = KT * CH // 2
            nc.scalar.dma_start(w_sb[:, 0:H], WP[:, 0:H])
            nc.sync.dma_start(x_sb[:, 0:H], xP[:, 0:H])
            nc.scalar.dma_start(w_sb[:, H:2 * H], WP[:, H:2 * H])
            nc.sync.dma_start(x_sb[:, H:2 * H], xP[:, H:2 * H])
            nc.gpsimd.dma_start(v_t[0][:], vP[:, 0:KT * CH])
            nc.scalar.dma_start(w_sb[:, KT * N:], WP[:, KT * N:])
            nc.sync.dma_start(x_sb[:, KT * CH:], xP[:, KT * CH:])
            nc.gpsimd.dma_start(v_t[1][:], vP[:, KT * CH:])
            bias_sb = wpool.tile([P, MT], f32, tag="bias", name="bias_sb")
            nc.sync.dma_start(bias_sb[:], bias.rearrange("(m p) -> p m", p=P))

            wt = [w_sb[:, j * N:(j + 1) * N] for j in range(KT)]
            wb01 = [w_sb[:, (KT + j) * N:(KT + j + 1) * N] for j in range(KT)]

            def xs(c, j):
                return x_sb[:, (c * KT + j) * CH:(c * KT + j + 1) * CH]

            def ss(c, m):
                return v_t[c][:, m * CH:(m + 1) * CH]

            # one PSUM bank per (m-tile, chunk) so chunk hazards stay
            # independent under tile-granular tracking
            ps = [[psump.tile([P, CH], f32, tag=f"ps{m}_{c}",
                              name=f"ps{m}_{c}")
                   for c in range(NCH)] for m in range(MT)]

            # HAM warm-up (short: must not delay the first real matmul)
            for r in range(10):
                nc.tensor.matmul(
                    ps[r % MT][0][:],
                    lhsT=junk[:, 0:P], rhs=junk[:, 0:CH],
                    start=True, stop=True, skip_group_check=True,
                )

            def mm_round(weights, rhs_of_j, c, start, stop):
                for j in range(KT):
                    for m in range(MT):
                        nc.tensor.matmul(
                            ps[m][c][:],
                            lhsT=weights[j][:, m * P:(m + 1) * P],
                            rhs=rhs_of_j(j),
                            start=(start and j == 0),
                            stop=(stop and j == KT - 1),
                            skip_group_check=True,
                        )

            # init: psum = x @ Wt + v0 @ (0.1*Wb)   (== x@Wt + s0@Wb)
            for c in range(NCH):
                mm_round(wt, lambda j: xs(c, j), c, start=True, stop=False)
                mm_round(wb01, lambda j: ss(c, j), c, start=False, stop=False)
                # rescale state tiles in place: s = 0.1*v0 (runs on DVE
                # under the init matmuls; WAR dep keeps it after the reads)
                for m in range(MT):
                    nc.vector.tensor_scalar_mul(ss(c, m), ss(c, m), DT)

            # ---- unfolds: chunks alternate on the PE -------------------
            u_tiles = {}
            for k in range(UNFOLDS):
                last = k == UNFOLDS - 1
                for c in range(NCH):
                    f_t = [fpool.tile([P, CH], f16, tag=f"f{c}_{m}",
                                      name=f"f{k}_{c}_{m}")
                           for m in range(MT)]
                    if not last:
                        tmp_t = [tmpp.tile([P, CH], f16, tag=f"t{c}_{m}",
                                           name=f"t{k}_{c}_{m}")
                                 for m in range(MT)]
                        for m in range(MT):
                            # f = tanh(psum + bias), fp16 out feeds the PE
                            nc.scalar.activation(
                                f_t[m][:], ps[m][c][:],
                                TANH, bias=bias_sb[:, m:m + 1], scale=1.0,
                            )
                            # tmp = f - s   (fp16 2x, on the critical path)
                            nc.vector.tensor_tensor(
                                tmp_t[m][:], f_t[m][:], ss(c, m), SUB)
                        # psum += tmp @ (0.1*Wb)
                        mm_round(wb01, lambda j: tmp_t[j][:], c,
                                 start=False, stop=(k == UNFOLDS - 2))
                        # s += 0.1*tmp, decomposed into 2x-mode ops (stt
                        # is Pool-illegal and only 1x on DVE): scale on
                        # DVE, the adds mostly on GpSimd. All off the
                        # critical path (next use is a full round away).
                        t01 = [tmpp.tile([P, CH], f16, tag=f"t01{c}_{m}",
                                         name=f"t01{k}_{c}_{m}")
                               for m in range(MT)]
                        for m in range(MT):
                            nc.vector.tensor_scalar_mul(
                                t01[m][:], tmp_t[m][:], DT)
                        for m in range(MT):
                            eng = nc.vector if m == 0 else nc.gpsimd
                            eng.tensor_tensor(ss(c, m), ss(c, m),
                                              t01[m][:], ADD)
                        if k == UNFOLDS - 2:
                            # precompute u = 0.9*s_5 under the last round
                            for m in range(MT):
                                u_t = outp.tile([P, CH], f16,
                                                tag=f"u{c}_{m}",
                                                name=f"u{c}_{m}")
                                nc.vector.tensor_scalar_mul(
                                    u_t[:], ss(c, m), 1.0 - DT)
                                u_tiles[(c, m)] = u_t
                    else:
                        # final unfold: s_out = 0.1*f + u
                        for m in range(MT):
                            nc.scalar.activation(
                                f_t[m][:], ps[m][c][:],
                                TANH, bias=bias_sb[:, m:m + 1], scale=1.0,
                            )
                            o_t = outp.tile([P, CH], f32, tag=f"o{c}_{m}",
                                            name=f"o{c}_{m}")
                            nc.vector.scalar_tensor_tensor(
                                o_t[:], f_t[m][:], DT, u_tiles[(c, m)][:],
                                op0=MULT, op1=ADD,
                            )
                            out_eng = ((nc.sync, nc.scalar, nc.gpsimd,
                                        nc.sync) if c == 0 else
                                       (nc.scalar, nc.gpsimd, nc.sync,
                                        nc.gpsimd))[m]
                            out_eng.dma_start(
                                outT[m * P:(m + 1) * P, c * CH:(c + 1) * CH],
                                o_t[:])

    nc.compile()
    return nc


def _get_nc():
    global _compiled_nc
    if _compiled_nc is None:
        _compiled_nc = _build_nc()
    return _compiled_nc


def _pack_cm(a):
    """(512, 1024) f32 -> (128, NCH*KT*CH) fp16, chunk-major (c, j)."""
    t = a.reshape(KT, P, NCH, CH).transpose(1, 2, 0, 3).reshape(P, -1)
    return np.ascontiguousarray(t).astype(np.float16)


def make_in_maps(x, s, W, b):
    xT = np.ascontiguousarray(x.T)           # (D, B)
    sT = np.ascontiguousarray(s.T)           # (N, B)
    Wt = W[:D].reshape(KT, P, N).transpose(1, 0, 2).reshape(P, -1)
    Wb01 = (DT * W[D:]).reshape(KT, P, N).transpose(1, 0, 2).reshape(P, -1)
    WPh = np.ascontiguousarray(
        np.concatenate([Wt, Wb01], axis=1)).astype(np.float16)
    in_maps = []
    for c in range(NCORES):
        sl = slice(c * BC, (c + 1) * BC)
        in_maps.append({
            "xP": _pack_cm(xT[:, sl]),
            "vP": _pack_cm(10.0 * sT[:, sl]),
            "WP": WPh,
            "bias": b,
        })
    return in_maps


def kernel(**inputs):
    from concourse.bass_utils import run_bass_kernel_spmd

    x = np.asarray(inputs["inputs"], dtype=np.float32)
    s = np.asarray(inputs["state"], dtype=np.float32)
    W = np.ascontiguousarray(np.asarray(inputs["W"], dtype=np.float32))
    b = np.ascontiguousarray(np.asarray(inputs["bias"], dtype=np.float32))

    in_maps = make_in_maps(x, s, W, b)
    nc = _get_nc()
    res = run_bass_kernel_spmd(nc, in_maps, list(range(NCORES))).results
    outT = np.concatenate([res[c]["outT"] for c in range(NCORES)], axis=1)
    out = np.ascontiguousarray(outT.T).astype(np.float32)
    return (out, out)


# revision 12
# speedup vs baseline: 1.1925x; 1.1925x over previous
"""CTRNN cell (6 Euler unfolds) on 8 Trainium2 NeuronCores.

Math (per unfold, 6x):
    f     = tanh([x, s] @ W + b)
    s_new = s + 0.1 * (-s + f)  = 0.9*s + 0.1*f

Strategy (v4):
  - Data-parallel over batch: B=8192 -> 1024 rows/core, no cross-core
    communication. Host does the cheap numpy transposes/packing/casts.
  - All tensors kept TRANSPOSED on-chip (feature dim on SBUF partitions,
    batch on the free dim): state/x feed the tensor engine as the moving
    operand and W k-tile slices are directly the stationary lhsT.
  - Everything fp16 (except PSUM + final output f32). fp16 runs the PE
    at bf16 rate, gives 16-bit DVE ops the 2x packed perf mode, and its
    10-bit mantissa keeps the 5 accumulating state updates accurate:
    simulated end-to-end rel err 1.2e-3 (gate is 2e-2).
  - Delta-form matmuls: one persistent PSUM bank per (m-tile, chunk)
    holds z = x@Wt + s_k@Wb across all unfolds, updated with
    psum += (f_k - s_k) @ (0.1*Wb). 7-logical-matmul FLOP floor; PSUM
    never restarts. 0.1*Wb is host-prefolded. The init matmul feeds
    v0 = 10*s0 through 0.1*Wb (== s0@Wb), then the state tiles are
    rescaled in place to s (one free 2x DVE pass under the init
    matmuls) so the hot loop's critical op is the plain fp16 subtract
    tmp = f - s (2x mode, 408ns). The state update s += 0.1*tmp is a
    1x scalar_tensor_tensor but runs off the critical path, split
    2x DVE / 2x GpSimd per round.
  - Batch is split into 2 chunks of 512 that alternate on the PE: while
    chunk A runs its 16 delta matmuls, chunk B does tanh (ACT) + the
    tmp subtract (DVE) for the same unfold, and vice versa. The PE
    never waits out the ACT->DVE chain, so it stays busy (and the HAM
    clock gate stays at 2.4 GHz) through the steady state. All
    PSUM/state tiles are split per (m-tile, chunk) so Tile's
    tile-granular hazard tracking cannot serialize the two chunks.
  - Tail: u = 0.9*s_5 is precomputed under the last matmul round, so
    the final unfold is one fused op out = 0.1*f + u (split DVE/GpSimd)
    feeding 8 output DMAs balanced over the sync/scalar/gpsimd queues.
  - Inputs are host-packed (128, k*4KB) chunk-major so every DMA has
    >=2KB-contiguous per-partition runs, spread over the sync/scalar
    HWDGE rings and the gpsimd SWDGE path; the first x/W pieces are
    split so the init matmuls can start as early as possible.
  - A short junk-matmul warm-up bridges the PE from the NEFF preamble
    to the first data landing without delaying the real matmuls.
"""

import numpy as np

UNFOLDS = 6
DT = 0.1
B, D, N = 8192, 512, 512
NCORES = 8
BC = B // NCORES          # batch rows per core (1024)
CH = 512                  # chunk free-dim (one PSUM bank of f32)
NCH = BC // CH            # 2 chunks
P = 128
KT = D // P               # 4 k-tiles per operand half
MT = N // P               # 4 m-tiles of the output dim

_compiled_nc = None


def _build_nc():
    import concourse.bass as bass  # noqa: F401
    import concourse.bacc as bacc
    import concourse.tile as tile
    from concourse import mybir

    f32 = mybir.dt.float32
    f16 = mybir.dt.float16
    MULT = mybir.AluOpType.mult
    ADD = mybir.AluOpType.add
    SUB = mybir.AluOpType.subtract
    TANH = mybir.ActivationFunctionType.Tanh

    nc = bacc.Bacc("TRN2", target_bir_lowering=False, debug=False)

    xP = nc.dram_tensor("xP", [P, NCH * KT * CH], f16, kind="ExternalInput").ap()
    vP = nc.dram_tensor("vP", [P, NCH * KT * CH], f16, kind="ExternalInput").ap()
    WP = nc.dram_tensor("WP", [P, 2 * KT * N], f16, kind="ExternalInput").ap()
    bias = nc.dram_tensor("bias", [N], f32, kind="ExternalInput").ap()
    outT = nc.dram_tensor("outT", [N, BC], f32, kind="ExternalOutput").ap()

    with tile.TileContext(nc) as tc:
        with (
            tc.tile_pool(name="wpool", bufs=1) as wpool,
            tc.tile_pool(name="data", bufs=1) as data,
            tc.tile_pool(name="fpool", bufs=2) as fpool,
            tc.tile_pool(name="tmpp", bufs=2) as tmpp,
            tc.tile_pool(name="outp", bufs=1) as outp,
            tc.tile_pool(name="psum", bufs=1, space="PSUM") as psump,
        ):
            # warm-up junk tile first thing on the gpsimd queue so the
            # warm-up matmuls can start before any data lands
            junk = wpool.tile([P, N], f16, tag="junk", name="junk")
            nc.gpsimd.memset(junk[:], 0)

            # ---- input DMAs --------------------------------------------
            # first-needed first, split fine so the init matmuls start
            # as early as possible: Wt + x_A halves feed init chunk A.
            w_sb = wpool.tile([P, 2 * KT * N], f16, tag="w", name="w_sb")
            x_sb = data.tile([P, NCH * KT * CH], f16, tag="x", name="x_sb")
            v_t = [data.tile([P, KT * CH], f16, tag=f"v{c}", name=f"v{c}")
                   for c in range(NCH)]
            H = KT * CH // 2
            nc.scalar.dma_start(w_sb[:, 0:H], WP[:, 0:H])
            nc.sync.dma_start(x_sb[:, 0:H], xP[:, 0:H])
            nc.gpsimd.dma_start(v_t[0][:, 0:H], vP[:, 0:H])
            nc.scalar.dma_start(w_sb[:, H:2 * H], WP[:, H:2 * H])
            nc.sync.dma_start(x_sb[:, H:2 * H], xP[:, H:2 * H])
            nc.gpsimd.dma_start(v_t[0][:, H:2 * H], vP[:, H:2 * H])
            nc.scalar.dma_start(w_sb[:, 2 * H:3 * H], WP[:, 2 * H:3 * H])
            nc.scalar.dma_start(w_sb[:, 3 * H:4 * H], WP[:, 3 * H:4 * H])
            nc.sync.dma_start(x_sb[:, KT * CH:], xP[:, KT * CH:])
            nc.gpsimd.dma_start(v_t[1][:], vP[:, KT * CH:])
            bias_sb = wpool.tile([P, MT], f32, tag="bias", name="bias_sb")
            nc.sync.dma_start(bias_sb[:], bias.rearrange("(m p) -> p m", p=P))

            wt = [w_sb[:, j * N:(j + 1) * N] for j in range(KT)]
            wb01 = [w_sb[:, (KT + j) * N:(KT + j + 1) * N] for j in range(KT)]

            def xs(c, j):
                return x_sb[:, (c * KT + j) * CH:(c * KT + j + 1) * CH]

            def ss(c, m):
                return v_t[c][:, m * CH:(m + 1) * CH]

            # one PSUM bank per (m-tile, chunk) so chunk hazards stay
            # independent under tile-granular tracking
            ps = [[psump.tile([P, CH], f32, tag=f"ps{m}_{c}",
                              name=f"ps{m}_{c}")
                   for c in range(NCH)] for m in range(MT)]

            # HAM warm-up (short: must not delay the first real matmul)
            for r in range(5):
                nc.tensor.matmul(
                    ps[r % MT][0][:],
                    lhsT=junk[:, 0:P], rhs=junk[:, 0:CH],
                    start=True, stop=True, skip_group_check=True,
                )

            def mm_round(weights, rhs_of_j, c, start, stop):
                for j in range(KT):
                    for m in range(MT):
                        nc.tensor.matmul(
                            ps[m][c][:],
                            lhsT=weights[j][:, m * P:(m + 1) * P],
                            rhs=rhs_of_j(j),
                            start=(start and j == 0),
                            stop=(stop and j == KT - 1),
                            skip_group_check=True,
                        )

            # init: psum = x @ Wt + v0 @ (0.1*Wb)   (== x@Wt + s0@Wb)
            for c in range(NCH):
                mm_round(wt, lambda j: xs(c, j), c, start=True, stop=False)
                mm_round(wb01, lambda j: ss(c, j), c, start=False, stop=False)
                # rescale state tiles in place: s = 0.1*v0 (runs on DVE
                # under the init matmuls; WAR dep keeps it after the reads)
                for m in range(MT):
                    nc.vector.tensor_scalar_mul(ss(c, m), ss(c, m), DT)

            # ---- unfolds: chunks alternate on the PE -------------------
            u_tiles = {}
            for k in range(UNFOLDS):
                last = k == UNFOLDS - 1
                for c in range(NCH):
                    f_t = [fpool.tile([P, CH], f16, tag=f"f{c}_{m}",
                                      name=f"f{k}_{c}_{m}")
                           for m in range(MT)]
                    if not last:
                        tmp_t = [tmpp.tile([P, CH], f16, tag=f"t{c}_{m}",
                                           name=f"t{k}_{c}_{m}")
                                 for m in range(MT)]
                        for m in range(MT):
                            # f = tanh(psum + bias), fp16 out feeds the PE
                            nc.scalar.activation(
                                f_t[m][:], ps[m][c][:],
                                TANH, bias=bias_sb[:, m:m + 1], scale=1.0,
                            )
                            # tmp = f - s   (fp16 2x, on the critical path)
                            nc.vector.tensor_tensor(
                                tmp_t[m][:], f_t[m][:], ss(c, m), SUB)
                        # psum += tmp @ (0.1*Wb)
                        mm_round(wb01, lambda j: tmp_t[j][:], c,
                                 start=False, stop=(k == UNFOLDS - 2))
                        # s += 0.1*tmp, decomposed into 2x-mode ops (stt
                        # is Pool-illegal and only 1x on DVE): scale on
                        # DVE, the adds mostly on GpSimd. All off the
                        # critical path (next use is a full round away).
                        t01 = [tmpp.tile([P, CH], f16, tag=f"t01{c}_{m}",
                                         name=f"t01{k}_{c}_{m}")
                               for m in range(MT)]
                        for m in range(MT):
                            nc.vector.tensor_scalar_mul(
                                t01[m][:], tmp_t[m][:], DT)
                        # adds spread over DVE / GpSimd / DMA-CCE so no
                        # single engine eats the whole state update
                        nc.vector.tensor_tensor(ss(c, 0), ss(c, 0),
                                                t01[0][:], ADD)
                        nc.gpsimd.tensor_tensor(ss(c, 1), ss(c, 1),
                                                t01[1][:], ADD)
                        for m in (2, 3):
                            nc.gpsimd.dma_start(ss(c, m), t01[m][:],
                                                accum_op=ADD)
                        if k == UNFOLDS - 2:
                            # precompute u = 0.9*s_5 under the last round
                            for m in range(MT):
                                u_t = outp.tile([P, CH], f16,
                                                tag=f"u{c}_{m}",
                                                name=f"u{c}_{m}")
                                nc.vector.tensor_scalar_mul(
                                    u_t[:], ss(c, m), 1.0 - DT)
                                u_tiles[(c, m)] = u_t
                    else:
                        # final unfold: s_out = 0.1*f + u
                        for m in range(MT):
                            nc.scalar.activation(
                                f_t[m][:], ps[m][c][:],
                                TANH, bias=bias_sb[:, m:m + 1], scale=1.0,
                            )
                            o_t = outp.tile([P, CH], f32, tag=f"o{c}_{m}",
                                            name=f"o{c}_{m}")
                            nc.vector.scalar_tensor_tensor(
                                o_t[:], f_t[m][:], DT, u_tiles[(c, m)][:],
                                op0=MULT, op1=ADD,
                            )
                            out_eng = ((nc.sync, nc.scalar, nc.gpsimd,
                                        nc.sync) if c == 0 else
                                       (nc.scalar, nc.gpsimd, nc.sync,
                                        nc.gpsimd))[m]
                            out_eng.dma_start(
                                outT[m * P:(m + 1) * P, c * CH:(c + 1) * CH],
                                o_t[:])

    nc.compile()
    return nc


def _get_nc():
    global _compiled_nc
    if _compiled_nc is None:
        _compiled_nc = _build_nc()
    return _compiled_nc


def _pack_cm(a):
    """(512, 1024) f32 -> (128, NCH*KT*CH) fp16, chunk-major (c, j)."""
    t = a.reshape(KT, P, NCH, CH).transpose(1, 2, 0, 3).reshape(P, -1)
    return np.ascontiguousarray(t).astype(np.float16)


def make_in_maps(x, s, W, b):
    xT = np.ascontiguousarray(x.T)           # (D, B)
    sT = np.ascontiguousarray(s.T)           # (N, B)
    Wt = W[:D].reshape(KT, P, N).transpose(1, 0, 2).reshape(P, -1)
    Wb01 = (DT * W[D:]).reshape(KT, P, N).transpose(1, 0, 2).reshape(P, -1)
    WPh = np.ascontiguousarray(
        np.concatenate([Wt, Wb01], axis=1)).astype(np.float16)
    in_maps = []
    for c in range(NCORES):
        sl = slice(c * BC, (c + 1) * BC)
        in_maps.append({
            "xP": _pack_cm(xT[:, sl]),
            "vP": _pack_cm(10.0 * sT[:, sl]),
            "WP": WPh,
            "bias": b,
        })
    return in_maps


def kernel(**inputs):
    from concourse.bass_utils import run_bass_kernel_spmd

    x = np.asarray(inputs["inputs"], dtype=np.float32)
    s = np.asarray(inputs["state"], dtype=np.float32)
    W = np.ascontiguousarray(np.asarray(inputs["W"], dtype=np.float32))
    b = np.ascontiguousarray(np.asarray(inputs["bias"], dtype=np.float32))

    in_maps = make_in_maps(x, s, W, b)
    nc = _get_nc()
    res = run_bass_kernel_spmd(nc, in_maps, list(range(NCORES))).results
    outT = np.concatenate([res[c]["outT"] for c in range(NCORES)], axis=1)
    out = np.ascontiguousarray(outT.T).astype(np.float32)
    return (out, out)


# revision 15
# speedup vs baseline: 1.2009x; 1.0070x over previous
"""CTRNN cell (6 Euler unfolds) on 8 Trainium2 NeuronCores.

Math (per unfold, 6x):
    f     = tanh([x, s] @ W + b)
    s_new = s + 0.1 * (-s + f)  = 0.9*s + 0.1*f

Strategy (v4):
  - Data-parallel over batch: B=8192 -> 1024 rows/core, no cross-core
    communication. Host does the cheap numpy transposes/packing/casts.
  - All tensors kept TRANSPOSED on-chip (feature dim on SBUF partitions,
    batch on the free dim): state/x feed the tensor engine as the moving
    operand and W k-tile slices are directly the stationary lhsT.
  - Everything fp16 (except PSUM + final output f32). fp16 runs the PE
    at bf16 rate, gives 16-bit DVE ops the 2x packed perf mode, and its
    10-bit mantissa keeps the 5 accumulating state updates accurate:
    simulated end-to-end rel err 1.2e-3 (gate is 2e-2).
  - Delta-form matmuls: one persistent PSUM bank per (m-tile, chunk)
    holds z = x@Wt + s_k@Wb across all unfolds, updated with
    psum += (f_k - s_k) @ (0.1*Wb). 7-logical-matmul FLOP floor; PSUM
    never restarts. 0.1*Wb is host-prefolded. The init matmul feeds
    v0 = 10*s0 through 0.1*Wb (== s0@Wb), then the state tiles are
    rescaled in place to s (one free 2x DVE pass under the init
    matmuls) so the hot loop's critical op is the plain fp16 subtract
    tmp = f - s (2x mode, 408ns). The state update s += 0.1*tmp is a
    1x scalar_tensor_tensor but runs off the critical path, split
    2x DVE / 2x GpSimd per round.
  - Batch is split into 2 chunks of 512 that alternate on the PE: while
    chunk A runs its 16 delta matmuls, chunk B does tanh (ACT) + the
    tmp subtract (DVE) for the same unfold, and vice versa. The PE
    never waits out the ACT->DVE chain, so it stays busy (and the HAM
    clock gate stays at 2.4 GHz) through the steady state. All
    PSUM/state tiles are split per (m-tile, chunk) so Tile's
    tile-granular hazard tracking cannot serialize the two chunks.
  - Tail: u = 0.9*s_5 is precomputed under the last matmul round, so
    the final unfold is one fused op out = 0.1*f + u (split DVE/GpSimd)
    feeding 8 output DMAs balanced over the sync/scalar/gpsimd queues.
  - Inputs are host-packed (128, k*4KB) chunk-major so every DMA has
    >=2KB-contiguous per-partition runs, spread over the sync/scalar
    HWDGE rings and the gpsimd SWDGE path; the first x/W pieces are
    split so the init matmuls can start as early as possible.
  - A short junk-matmul warm-up bridges the PE from the NEFF preamble
    to the first data landing without delaying the real matmuls.
"""

import numpy as np

UNFOLDS = 6
DT = 0.1
B, D, N = 8192, 512, 512
NCORES = 8
BC = B // NCORES          # batch rows per core (1024)
CH = 512                  # chunk free-dim (one PSUM bank of f32)
NCH = BC // CH            # 2 chunks
P = 128
KT = D // P               # 4 k-tiles per operand half
MT = N // P               # 4 m-tiles of the output dim

_compiled_nc = None


def _build_nc():
    import concourse.bass as bass  # noqa: F401
    import concourse.bacc as bacc
    import concourse.tile as tile
    from concourse import mybir

    f32 = mybir.dt.float32
    f16 = mybir.dt.float16
    MULT = mybir.AluOpType.mult
    ADD = mybir.AluOpType.add
    SUB = mybir.AluOpType.subtract
    TANH = mybir.ActivationFunctionType.Tanh

    nc = bacc.Bacc("TRN2", target_bir_lowering=False, debug=False)

    xP = nc.dram_tensor("xP", [P, NCH * KT * CH], f16, kind="ExternalInput").ap()
    vP = nc.dram_tensor("vP", [P, NCH * KT * CH], f16, kind="ExternalInput").ap()
    WP = nc.dram_tensor("WP", [P, 2 * KT * N], f16, kind="ExternalInput").ap()
    bias = nc.dram_tensor("bias", [N], f32, kind="ExternalInput").ap()
    outT = nc.dram_tensor("outT", [N, BC], f32, kind="ExternalOutput").ap()

    with tile.TileContext(nc) as tc:
        with (
            tc.tile_pool(name="wpool", bufs=1) as wpool,
            tc.tile_pool(name="data", bufs=1) as data,
            tc.tile_pool(name="fpool", bufs=2) as fpool,
            tc.tile_pool(name="tmpp", bufs=2) as tmpp,
            tc.tile_pool(name="outp", bufs=1) as outp,
            tc.tile_pool(name="psum", bufs=1, space="PSUM") as psump,
        ):
            # warm-up junk tile first thing on the gpsimd queue so the
            # warm-up matmuls can start before any data lands
            junk = wpool.tile([P, N], f16, tag="junk", name="junk")
            nc.gpsimd.memset(junk[:], 0)

            # ---- input DMAs --------------------------------------------
            # first-needed first, split fine so the init matmuls start
            # as early as possible: Wt + x_A halves feed init chunk A.
            w_sb = wpool.tile([P, 2 * KT * N], f16, tag="w", name="w_sb")
            x_sb = data.tile([P, NCH * KT * CH], f16, tag="x", name="x_sb")
            v_t = [data.tile([P, KT * CH], f16, tag=f"v{c}", name=f"v{c}")
                   for c in range(NCH)]
            H = KT * CH // 2
            nc.scalar.dma_start(w_sb[:, 0:H], WP[:, 0:H])
            nc.sync.dma_start(x_sb[:, 0:H], xP[:, 0:H])
            nc.gpsimd.dma_start(v_t[0][:, 0:H], vP[:, 0:H])
            nc.scalar.dma_start(w_sb[:, H:2 * H], WP[:, H:2 * H])
            nc.sync.dma_start(x_sb[:, H:2 * H], xP[:, H:2 * H])
            nc.gpsimd.dma_start(v_t[0][:, H:2 * H], vP[:, H:2 * H])
            nc.scalar.dma_start(w_sb[:, 2 * H:3 * H], WP[:, 2 * H:3 * H])
            nc.scalar.dma_start(w_sb[:, 3 * H:4 * H], WP[:, 3 * H:4 * H])
            nc.sync.dma_start(x_sb[:, KT * CH:], xP[:, KT * CH:])
            nc.gpsimd.dma_start(v_t[1][:], vP[:, KT * CH:])
            bias_sb = wpool.tile([P, MT], f32, tag="bias", name="bias_sb")
            nc.sync.dma_start(bias_sb[:], bias.rearrange("(m p) -> p m", p=P))

            wt = [w_sb[:, j * N:(j + 1) * N] for j in range(KT)]
            wb01 = [w_sb[:, (KT + j) * N:(KT + j + 1) * N] for j in range(KT)]

            def xs(c, j):
                return x_sb[:, (c * KT + j) * CH:(c * KT + j + 1) * CH]

            def ss(c, m):
                return v_t[c][:, m * CH:(m + 1) * CH]

            # one PSUM bank per (m-tile, chunk) so chunk hazards stay
            # independent under tile-granular tracking
            ps = [[psump.tile([P, CH], f32, tag=f"ps{m}_{c}",
                              name=f"ps{m}_{c}")
                   for c in range(NCH)] for m in range(MT)]

            # HAM warm-up (short: must not delay the first real matmul)
            for r in range(8):
                nc.tensor.matmul(
                    ps[r % MT][0][:],
                    lhsT=junk[:, 0:P], rhs=junk[:, 0:CH],
                    start=True, stop=True, skip_group_check=True,
                )

            def mm_round(weights, rhs_of_j, c, start, stop):
                for j in range(KT):
                    for m in range(MT):
                        nc.tensor.matmul(
                            ps[m][c][:],
                            lhsT=weights[j][:, m * P:(m + 1) * P],
                            rhs=rhs_of_j(j),
                            start=(start and j == 0),
                            stop=(stop and j == KT - 1),
                            skip_group_check=True,
                        )

            # init: psum = x @ Wt + v0 @ (0.1*Wb)   (== x@Wt + s0@Wb)
            for c in range(NCH):
                mm_round(wt, lambda j: xs(c, j), c, start=True, stop=False)
                mm_round(wb01, lambda j: ss(c, j), c, start=False, stop=False)
                # rescale state tiles in place: s = 0.1*v0 (runs on DVE
                # under the init matmuls; WAR dep keeps it after the reads)
                for m in range(MT):
                    nc.vector.tensor_scalar_mul(ss(c, m), ss(c, m), DT)

            # ---- unfolds: chunks alternate on the PE -------------------
            u_tiles = {}
            for k in range(UNFOLDS):
                last = k == UNFOLDS - 1
                for c in range(NCH):
                    f_t = [fpool.tile([P, CH], f16, tag=f"f{c}_{m}",
                                      name=f"f{k}_{c}_{m}")
                           for m in range(MT)]
                    if not last:
                        tmp_t = [tmpp.tile([P, CH], f16, tag=f"t{c}_{m}",
                                           name=f"t{k}_{c}_{m}")
                                 for m in range(MT)]
                        for m in range(MT):
                            # f = tanh(psum + bias), fp16 out feeds the PE
                            nc.scalar.activation(
                                f_t[m][:], ps[m][c][:],
                                TANH, bias=bias_sb[:, m:m + 1], scale=1.0,
                            )
                            # tmp = f - s   (fp16 2x, on the critical path)
                            nc.vector.tensor_tensor(
                                tmp_t[m][:], f_t[m][:], ss(c, m), SUB)
                        # psum += tmp @ (0.1*Wb)
                        mm_round(wb01, lambda j: tmp_t[j][:], c,
                                 start=False, stop=(k == UNFOLDS - 2))
                        # s += 0.1*tmp, decomposed into 2x-mode ops (stt
                        # is Pool-illegal and only 1x on DVE): scale on
                        # DVE, the adds mostly on GpSimd. All off the
                        # critical path (next use is a full round away).
                        t01 = [tmpp.tile([P, CH], f16, tag=f"t01{c}_{m}",
                                         name=f"t01{k}_{c}_{m}")
                               for m in range(MT)]
                        for m in range(MT):
                            nc.vector.tensor_scalar_mul(
                                t01[m][:], tmp_t[m][:], DT)
                        # adds spread over DVE / GpSimd / DMA-CCE so no
                        # single engine eats the whole state update; the
                        # last delta round keeps them all on DVE so the
                        # u = 0.9*s_5 precompute isn't gated on slow CCE
                        # completions (the PE has no further rounds to
                        # feed, so the DVE overshoot there is free)
                        if k == UNFOLDS - 2:
                            for m in range(MT):
                                nc.vector.tensor_tensor(
                                    ss(c, m), ss(c, m), t01[m][:], ADD)
                        else:
                            nc.vector.tensor_tensor(ss(c, 0), ss(c, 0),
                                                    t01[0][:], ADD)
                            nc.gpsimd.tensor_tensor(ss(c, 1), ss(c, 1),
                                                    t01[1][:], ADD)
                            for m in (2, 3):
                                nc.gpsimd.dma_start(ss(c, m), t01[m][:],
                                                    accum_op=ADD)
                        if k == UNFOLDS - 2:
                            # precompute u = 0.9*s_5 under the last round
                            for m in range(MT):
                                u_t = outp.tile([P, CH], f16,
                                                tag=f"u{c}_{m}",
                                                name=f"u{c}_{m}")
                                nc.vector.tensor_scalar_mul(
                                    u_t[:], ss(c, m), 1.0 - DT)
                                u_tiles[(c, m)] = u_t
                    else:
                        # final unfold: s_out = 0.1*f + u
                        for m in range(MT):
                            nc.scalar.activation(
                                f_t[m][:], ps[m][c][:],
                                TANH, bias=bias_sb[:, m:m + 1], scale=1.0,
                            )
                            o_t = outp.tile([P, CH], f32, tag=f"o{c}_{m}",
                                            name=f"o{c}_{m}")
                            nc.vector.scalar_tensor_tensor(
                                o_t[:], f_t[m][:], DT, u_tiles[(c, m)][:],
                                op0=MULT, op1=ADD,
                            )
                            # never the scalar queue: the final tanh ACTs
                            # must not queue behind out-DMA issues
                            out_eng = ((nc.sync, nc.gpsimd, nc.sync,
                                        nc.gpsimd) if c == 0 else
                                       (nc.gpsimd, nc.sync, nc.gpsimd,
                                        nc.sync))[m]
                            out_eng.dma_start(
                                outT[m * P:(m + 1) * P, c * CH:(c + 1) * CH],
                                o_t[:])

    nc.compile()
    return nc


def _get_nc():
    global _compiled_nc
    if _compiled_nc is None:
        _compiled_nc = _build_nc()
    return _compiled_nc


def _pack_cm(a):
    """(512, 1024) f32 -> (128, NCH*KT*CH) fp16, chunk-major (c, j)."""
    t = a.reshape(KT, P, NCH, CH).transpose(1, 2, 0, 3).reshape(P, -1)
    return np.ascontiguousarray(t).astype(np.float16)


def make_in_maps(x, s, W, b):
    xT = np.ascontiguousarray(x.T)           # (D, B)
    sT = np.ascontiguousarray(s.T)           # (N, B)
    Wt = W[:D].reshape(KT, P, N).transpose(1, 0, 2).reshape(P, -1)
    Wb01 = (DT * W[D:]).reshape(KT, P, N).transpose(1, 0, 2).reshape(P, -1)
    WPh = np.ascontiguousarray(
        np.concatenate([Wt, Wb01], axis=1)).astype(np.float16)
    in_maps = []
    for c in range(NCORES):
        sl = slice(c * BC, (c + 1) * BC)
        in_maps.append({
            "xP": _pack_cm(xT[:, sl]),
            "vP": _pack_cm(10.0 * sT[:, sl]),
            "WP": WPh,
            "bias": b,
        })
    return in_maps


def kernel(**inputs):
    from concourse.bass_utils import run_bass_kernel_spmd

    x = np.asarray(inputs["inputs"], dtype=np.float32)
    s = np.asarray(inputs["state"], dtype=np.float32)
    W = np.ascontiguousarray(np.asarray(inputs["W"], dtype=np.float32))
    b = np.ascontiguousarray(np.asarray(inputs["bias"], dtype=np.float32))

    in_maps = make_in_maps(x, s, W, b)
    nc = _get_nc()
    res = run_bass_kernel_spmd(nc, in_maps, list(range(NCORES))).results
    outT = np.concatenate([res[c]["outT"] for c in range(NCORES)], axis=1)
    out = np.ascontiguousarray(outT.T).astype(np.float32)
    return (out, out)
